# revision 9
# baseline (speedup 1.0000x reference)
"""ColorDenseCRFLoss on 8 Trainium2 NeuronCores — 3-engine exp redesign.

Math: loss = -W/N * sum_n sum_ij K_ij S_ij, where for each image n
  K_ij = exp(-0.5*||f_i - f_j||^2)   (f = nearest-downsampled RGB / 15, P=4096 pts)
  S_ij = sum_k seg_k,i seg_k,j       (seg = bilinear-downsampled softmax, K=21)
Bilinear downsample at exactly 2x == 2x2 average pooling; nearest == stride-2.

Sharding: 2 cores per image (batch N=4 -> 8 cores). Symmetry via circulant
blocks: core h of image n owns row-blocks v=0..15 (of its rotated frame,
rotation 2048*h points) with column window d=0..16 (2176 cols); d=0 and d=16
columns are weighted 1/2 (folded into host-side segstk scaling) and the
grand total is doubled.

The PE emits pG = A*g + B' (A=128/ln2, B'=127*128-7; scale/bias folded in as
two extra contraction rows), so THREE engines convert PSUM tiles to K in
parallel:
 - ACT: exact exp via activation(Exp, scale=ln2/128, bias=-B'*ln2/128)
 - DVE: Schraudolph bf16: int16(max(pG,0)) bitcast as bf16 (RNE + saturation)
 - GpSimd: same via tensor_tensor(max vs zero tile) (tensor_scalar is slow on Q7)
The Schraudolph bias constant (-7) is calibrated so the kernel-weighted sum
error is ~3e-4 even if ALL elements used it; only ~half do.

G and AS matmuls are 4-way PE-tiled (row-tiles for G at partition offsets
0/32/64/96; column-tiles for AS at psum partition offsets) — all four streams
run concurrently on the PE. All seg staging (2x2 pool, transpose, window
stacking, edge halving) is done host-side; the device only does matmuls,
converts, and the DVE multiply-accumulate reduce.
"""

import sys

for _p in ("/opt/trn_rl_repo",):
    if _p not in sys.path:
        sys.path.insert(0, _p)

import numpy as np
import ml_dtypes

import bass_rust
import concourse.bass as bass
import concourse.mybir as mybir
from concourse.tile import TileContext
from concourse.bass_utils import run_bass_kernel_spmd

F32 = mybir.dt.float32
BF16 = mybir.dt.bfloat16
I16 = mybir.dt.int16

WEIGHT = 1e-7
SIGMA_RGB = 15.0
N_IMG = 4
P = 4096          # 64*64 points per image
WIN = 17 * 128    # d = 0..16 column window (2176)

LN2 = float(np.log(2.0))
A_SCALE = 128.0 / LN2            # Schraudolph exponent scale
C_ADJ = -7.0                     # chord-bias correction (calibrated)
B_BIAS = 127.0 * 128.0 + C_ADJ   # 16249
S_INV = LN2 / 128.0              # ACT inverse scale
BIAS_INV = -B_BIAS * LN2 / 128.0  # ACT inverse bias

# Per-group convert assignment: (step, tile, lo, hi, engine)
#   step: 0..3 = 512-col chunks, 4 = d16 (512-wide packed tile)
#   tile: 0 = kt1 (row-blocks q0/q1), 1 = kt2 (q2/q3); d16 has a single tile
#   engine: 'A' = ACT exact exp, 'V' = DVE schraudolph, 'G' = gpsimd schraudolph
ASSIGN = [
    (0, 0, 0, 1024, "A"),
    (0, 1, 0, 1024, "A"),
    (1, 0, 0, 1024, "A"),
    (1, 1, 0, 1024, "A"),
    (2, 0, 0, 1024, "A"),
    (2, 1, 0, 1024, "A"),
    (3, 0, 0, 1024, "V"),
    (3, 1, 0, 1024, "V"),
    (4, 0, 0, 0, "V"),
    (4, 1, 0, 0, "V"),
]

_CACHED = {}


def _pslice(t, lo, n, c0, c1):
    # [lo:lo+n, c0:c1] partition+col slice; base 96 must be expressed as a
    # double-slice (AP base_partition rejects 96).
    if lo >= 96:
        return t[64:128, c0:c1][lo - 64 : lo - 64 + n, :]
    return t[lo : lo + n, c0:c1]


def _build_nc():
    nc = bass.Bass(trn_type="TRN2", target_bir_lowering=False, debug=False)
    ab_d = nc.dram_tensor("ab", [4, 17, 512], BF16, kind="ExternalInput")
    bb_d = nc.dram_tensor("bb", [4, 17, 4096], BF16, kind="ExternalInput")
    st_d = nc.dram_tensor("segT", [128, 512], BF16, kind="ExternalInput")
    sk_d = nc.dram_tensor("segstk", [4, 128, WIN], BF16, kind="ExternalInput")
    out_d = nc.dram_tensor("acc", [128, 32], F32, kind="ExternalOutput")

    EXP = mybir.ActivationFunctionType.Exp
    MULT = mybir.AluOpType.mult
    MAX = mybir.AluOpType.max

    with TileContext(nc) as tc:
        with (
            tc.tile_pool(name="const", bufs=1) as constp,
            tc.tile_pool(name="kt", bufs=8) as ktp,
            tc.tile_pool(name="sc", bufs=2) as scp,
            tc.tile_pool(name="pg", bufs=3, space="PSUM") as pgp,
            tc.tile_pool(name="stk", bufs=2, space="PSUM") as stkp,
        ):
            bias_inv = constp.tile([128, 1], F32, tag="biasi")
            nc.vector.memset(bias_inv[:], BIAS_INV)
            zcol = constp.tile([128, 1024], BF16, tag="zcol")
            nc.vector.memset(zcol[:], 0.0)
            warm = constp.tile([128, 1], F32, tag="warm")
            # one-time ACT exp-table load starts immediately
            nc.scalar.activation(warm[:], bias_inv[:], EXP, bias=bias_inv[:])

            ab = constp.tile([128, 512], BF16, tag="ab")
            bb = constp.tile([128, 4096], BF16, tag="bb")
            segT = constp.tile([128, 512], BF16, tag="segT")
            segstk = []
            for g in range(4):
                sktile = constp.tile([128, WIN], BF16, tag=f"segstk{g}")
                segstk.append(sktile)
            accT = constp.tile([128, 32], F32, tag="accT")
            nc.gpsimd.memset(accT[:], 0.0)

            # Parallel input staging across the three DMA-capable queues.
            # ACT gets only two issues so its converts start early; sync (no
            # other work) carries most; gpsimd's first convert is ~chunk 3.
            nc.sync.dma_start(_pslice(bb, 0, 17, 0, 4096), bb_d.ap()[0])
            nc.scalar.dma_start(_pslice(bb, 32, 17, 0, 4096), bb_d.ap()[1])
            nc.gpsimd.dma_start(_pslice(bb, 64, 17, 0, 4096), bb_d.ap()[2])
            nc.sync.dma_start(_pslice(bb, 96, 17, 0, 4096), bb_d.ap()[3])
            nc.sync.dma_start(_pslice(ab, 0, 17, 0, 512), ab_d.ap()[0])
            nc.scalar.dma_start(_pslice(ab, 32, 17, 0, 512), ab_d.ap()[1])
            nc.gpsimd.dma_start(_pslice(ab, 64, 17, 0, 512), ab_d.ap()[2])
            nc.gpsimd.dma_start(_pslice(ab, 96, 17, 0, 512), ab_d.ap()[3])
            nc.sync.dma_start(segT[:], st_d.ap())
            nc.sync.dma_start(segstk[0][:], sk_d.ap()[0])
            nc.gpsimd.dma_start(segstk[1][:], sk_d.ap()[1])
            nc.gpsimd.dma_start(segstk[2][:], sk_d.ap()[2])
            nc.sync.dma_start(segstk[3][:], sk_d.ap()[3])

            def emit_G(g, c):
                """4-way row-tiled G matmuls for chunk c of group g.

                Returns (kt1, kt2) after emitting converts per ASSIGN."""
                pg1 = pgp.tile([128, 1024], F32, tag="pg", name="pg1")
                pg2 = pgp.tile([128, 1024], F32, tag="pg", name="pg2")
                for q in range(4):
                    v = 4 * g + q
                    if c < 4:
                        lo = 128 * v + 512 * c
                        n = 512
                        pt = pg1 if q < 2 else pg2
                        off = 512 * (q % 2)
                    else:
                        # d16: full-partition writers must sit in distinct
                        # PSUM banks: q0/q1 -> pg1 cols 0/512, q2/q3 -> pg2
                        lo = 128 * v + 2048
                        n = 128
                        pt = pg1 if q < 2 else pg2
                        off = 512 * (q % 2)
                    nc.tensor.matmul(
                        pt[:, off : off + n],
                        _pslice(ab, 32 * q, 17, 128 * g, 128 * g + 128),
                        _pslice(bb, 32 * q, 17, lo, lo + n),
                        start=True,
                        stop=True,
                        tile_position=(32 * q, 0),
                    )
                kt1 = ktp.tile([128, 1024], BF16, tag="kt", name="kt1")
                kt2 = ktp.tile([128, 1024], BF16, tag="kt", name="kt2")
                kts = {0: (kt1, pg1), 1: (kt2, pg2)}
                for step, tile, lo, hi, eng in ASSIGN:
                    if (c < 4 and step != c) or (c == 4 and step != 4):
                        continue
                    kt, pt = kts[tile]
                    if c == 4:
                        # strided view over cols {0:128, 512:640}
                        ov = kt[:].bitcast(I16).rearrange(
                            "p (a b) -> p a b", b=512
                        )[:, :, 0:128]
                        iv = pt[:].rearrange("p (a b) -> p a b", b=512)[:, :, 0:128]
                        nc.vector.tensor_scalar(ov, iv, 0.0, None, MAX)
                        continue
                    if eng == "A":
                        nc.scalar.activation(
                            kt[:, lo:hi], pt[:, lo:hi], EXP,
                            bias=bias_inv[:], scale=S_INV,
                        )
                    elif eng == "V":
                        nc.vector.tensor_scalar(
                            kt[:, lo:hi].bitcast(I16), pt[:, lo:hi],
                            0.0, None, MAX,
                        )
                return kt1, kt2

            def emit_AS(g, c, kt1, kt2):
                """4-way column-tiled AS matmuls + DVE reduce for chunk c."""
                n = 512 if c < 4 else 128
                stk = stkp.tile([128, 512], F32, tag="stk")
                for q in range(4):
                    kt = kt1 if q < 2 else kt2
                    if c < 4:
                        rhs = kt[:, 512 * (q % 2) : 512 * (q % 2) + 512]
                        out = _pslice(stk, 32 * q, 32, 0, 512)
                    else:
                        rhs = kt[:, 512 * (q % 2) : 512 * (q % 2) + 128]
                        out = _pslice(stk, 32 * q, 32, 0, 128)
                    nc.tensor.matmul(
                        out,
                        segT[:, 32 * (4 * g + q) : 32 * (4 * g + q) + 32],
                        rhs,
                        start=True,
                        stop=True,
                        tile_position=(0, 32 * q),
                    )
                sct = scp.tile([128, 512], BF16, tag="sct")
                w0 = 512 * c if c < 4 else 2048
                nc.vector.scalar_tensor_tensor(
                    out=sct[:, 0:n],
                    in0=stk[:, 0:n],
                    scalar=1.0,
                    in1=segstk[g][:, w0 : w0 + n],
                    op0=MULT,
                    op1=MULT,
                    accum_out=accT[:, 8 * g + c : 8 * g + c + 1],
                )

            # Software-pipelined schedule: PE runs one G chunk ahead of AS.
            prev = None  # (g, c, kt1, kt2)
            for g in range(4):
                for c in range(5):
                    kts = emit_G(g, c)
                    if prev is not None:
                        emit_AS(prev[0], prev[1], prev[2], prev[3])
                    prev = (g, c) + kts
            emit_AS(prev[0], prev[1], prev[2], prev[3])

            nc.sync.dma_start(out_d.ap(), accT[:])
    _split_multiwait(nc)
    return nc


def _split_multiwait(nc):
    """walrus encodes at most one semaphore wait per instruction; hoist all
    but one wait onto standalone EventSemaphore instructions placed just
    before the instruction on the same engine queue."""
    ctr = 0
    for f in nc.m.functions:
        for blk in f.blocks:
            insts = blk.instructions
            out = []
            for inst in insts:
                si = inst.sync_info
                if si is not None and len(si.on_wait) > 1:
                    waits = list(si.on_wait)
                    for w in waits[:-1]:
                        es = mybir.InstEventSemaphore(
                            name=f"WSPLIT-{ctr}", ins=[], outs=[]
                        )
                        ctr += 1
                        es.engine = inst.engine
                        es.sync_info = bass_rust.SyncInfo(on_wait=[w], on_update=[])
                        out.append(es)
                    inst.sync_info = bass_rust.SyncInfo(
                        on_wait=[waits[-1]], on_update=list(si.on_update)
                    )
                out.append(inst)
            insts[:] = out


def _host_prep(images, segmentations):
    bf = ml_dtypes.bfloat16
    in_maps = []
    for cidx in range(8):
        n, h = cidx // 2, cidx % 2
        img = images[n][:, ::2, ::2]                       # nearest resize
        img = np.roll(img, -32 * h, axis=1).reshape(3, P)  # circulant rotation
        f = (img / SIGMA_RGB).astype(np.float64)
        f = f - f.mean(axis=1, keepdims=True)              # d2-invariant centering
        sq = (f * f).sum(axis=0)
        ones = np.ones((1, P), np.float64)
        b5 = np.concatenate([f, ones, (-0.5 * sq)[None]], axis=0)
        a5 = np.concatenate([f, (-0.5 * sq)[None], ones], axis=0)

        asc = A_SCALE * a5
        ah = asc.astype(bf)
        al = (asc - ah.astype(np.float64)).astype(bf)
        bh = b5.astype(bf)
        bl = (b5 - bh.astype(np.float64)).astype(bf)

        # ab[q]: rows 0-4 asc_hi, 5-9 asc_lo, 10-14 asc_hi, 15 = 127, 16 = 1
        # cols 128g..128g+127 hold row-block v=4g+q (points 128v..128v+127)
        ab = np.zeros((4, 17, 512), dtype=bf)
        for q in range(4):
            for g in range(4):
                v = 4 * g + q
                cg = slice(128 * g, 128 * g + 128)
                pv = slice(128 * v, 128 * v + 128)
                ab[q, 0:5, cg] = ah[:, pv]
                ab[q, 5:10, cg] = al[:, pv]
                ab[q, 10:15, cg] = ah[:, pv]
            ab[q, 15, :] = bf(127.0)
            ab[q, 16, :] = bf(1.0)
        # bb[q]: rows 0-4 b_hi, 5-9 b_hi, 10-14 b_lo, 15 = 128, 16 = -7
        b17 = np.zeros((17, 4096), dtype=bf)
        b17[0:5] = bh
        b17[5:10] = bh
        b17[10:15] = bl
        b17[15, :] = bf(128.0)
        b17[16, :] = bf(C_ADJ)
        bb = np.ascontiguousarray(np.broadcast_to(b17, (4, 17, 4096)))

        # seg: roll + 2x2 sum pool (/16 folded into final host scale)
        segr = np.roll(segmentations[n], -64 * h, axis=1).astype(np.float64)
        sp = segr.reshape(21, 64, 2, 64, 2).sum(axis=(2, 4)).reshape(21, P)
        spb = sp.astype(bf)

        segT = np.zeros((128, 512), dtype=bf)
        sT = segT.reshape(128, 16, 32)
        for v in range(16):
            sT[:, v, 0:21] = spb[:, 128 * v : 128 * v + 128].T
        segstk = np.zeros((4, 128, WIN), dtype=np.float32)
        for g in range(4):
            for q in range(4):
                v = 4 * g + q
                segstk[g, 32 * q : 32 * q + 21, :] = sp[:, 128 * v : 128 * v + WIN]
        segstk[:, :, 0:128] *= 0.5       # d=0 diagonal block half-weight
        segstk[:, :, 2048:WIN] *= 0.5    # d=16 symmetric-boundary half-weight
        segstk = segstk.astype(bf)

        in_maps.append(
            {
                "ab": ab,
                "bb": bb,
                "segT": np.ascontiguousarray(segT),
                "segstk": np.ascontiguousarray(segstk),
            }
        )
    return in_maps


def run(images, segmentations, trace=False):
    if "nc" not in _CACHED:
        _CACHED["nc"] = _build_nc()
    nc = _CACHED["nc"]
    in_maps = _host_prep(np.asarray(images), np.asarray(segmentations))
    res = run_bass_kernel_spmd(nc, in_maps, list(range(8)), trace=trace)
    total = np.float64(0.0)
    for r in res.results:
        acc = r["acc"].astype(np.float64)
        for g in range(4):
            total += acc[:, 8 * g : 8 * g + 5].sum()
    # x2 symmetric halves, /16 unscaled 2x2 pool (quadratic), -W, /N batch mean
    loss = -WEIGHT * 2.0 * total / 16.0 / N_IMG
    return np.array([loss], dtype=np.float32), res


def kernel(images, segmentations):
    out, _ = run(images, segmentations, trace=False)
    return out


# revision 11
# speedup vs baseline: 1.0008x; 1.0008x over previous
"""ColorDenseCRFLoss on 8 Trainium2 NeuronCores — 3-engine exp redesign.

Math: loss = -W/N * sum_n sum_ij K_ij S_ij, where for each image n
  K_ij = exp(-0.5*||f_i - f_j||^2)   (f = nearest-downsampled RGB / 15, P=4096 pts)
  S_ij = sum_k seg_k,i seg_k,j       (seg = bilinear-downsampled softmax, K=21)
Bilinear downsample at exactly 2x == 2x2 average pooling; nearest == stride-2.

Sharding: 2 cores per image (batch N=4 -> 8 cores). Symmetry via circulant
blocks: core h of image n owns row-blocks v=0..15 (of its rotated frame,
rotation 2048*h points) with column window d=0..16 (2176 cols); d=0 and d=16
columns are weighted 1/2 (folded into host-side segstk scaling) and the
grand total is doubled.

The PE emits pG = A*g + B' (A=128/ln2, B'=127*128-7; scale/bias folded in as
two extra contraction rows), so THREE engines convert PSUM tiles to K in
parallel:
 - ACT: exact exp via activation(Exp, scale=ln2/128, bias=-B'*ln2/128)
 - DVE: Schraudolph bf16: int16(max(pG,0)) bitcast as bf16 (RNE + saturation)
 - GpSimd: same via tensor_tensor(max vs zero tile) (tensor_scalar is slow on Q7)
The Schraudolph bias constant (-7) is calibrated so the kernel-weighted sum
error is ~3e-4 even if ALL elements used it; only ~half do.

G and AS matmuls are 4-way PE-tiled (row-tiles for G at partition offsets
0/32/64/96; column-tiles for AS at psum partition offsets) — all four streams
run concurrently on the PE. All seg staging (2x2 pool, transpose, window
stacking, edge halving) is done host-side; the device only does matmuls,
converts, and the DVE multiply-accumulate reduce.
"""

import sys

for _p in ("/opt/trn_rl_repo",):
    if _p not in sys.path:
        sys.path.insert(0, _p)

import numpy as np
import ml_dtypes

import bass_rust
import concourse.bass as bass
import concourse.mybir as mybir
from concourse.tile import TileContext
from concourse.bass_utils import run_bass_kernel_spmd

F32 = mybir.dt.float32
BF16 = mybir.dt.bfloat16
I16 = mybir.dt.int16

WEIGHT = 1e-7
SIGMA_RGB = 15.0
N_IMG = 4
P = 4096          # 64*64 points per image
WIN = 17 * 128    # d = 0..16 column window (2176)

LN2 = float(np.log(2.0))
A_SCALE = 128.0 / LN2            # Schraudolph exponent scale
C_ADJ = -7.0                     # chord-bias correction (calibrated)
B_BIAS = 127.0 * 128.0 + C_ADJ   # 16249
S_INV = LN2 / 128.0              # ACT inverse scale
BIAS_INV = -B_BIAS * LN2 / 128.0  # ACT inverse bias

# Per-group convert assignment: (step, tile, lo, hi, engine)
#   step: 0..3 = 512-col chunks, 4 = d16 (512-wide packed tile)
#   tile: 0 = kt1 (row-blocks q0/q1), 1 = kt2 (q2/q3); d16 has a single tile
#   engine: 'A' = ACT exact exp, 'V' = DVE schraudolph, 'G' = gpsimd schraudolph
ASSIGN = [
    (0, 0, 0, 1024, "A"),
    (0, 1, 0, 1024, "A"),
    (1, 0, 0, 1024, "A"),
    (1, 1, 0, 1024, "A"),
    (2, 0, 0, 1024, "A"),
    (2, 1, 0, 1024, "A"),
    (3, 0, 0, 1024, "V"),
    (3, 1, 0, 1024, "V"),
    (4, 0, 0, 0, "V"),
    (4, 1, 0, 0, "V"),
]

_CACHED = {}


def _pslice(t, lo, n, c0, c1):
    # [lo:lo+n, c0:c1] partition+col slice; base 96 must be expressed as a
    # double-slice (AP base_partition rejects 96).
    if lo >= 96:
        return t[64:128, c0:c1][lo - 64 : lo - 64 + n, :]
    return t[lo : lo + n, c0:c1]


def _build_nc():
    nc = bass.Bass(trn_type="TRN2", target_bir_lowering=False, debug=False)
    ab_d = nc.dram_tensor("ab", [4, 17, 512], BF16, kind="ExternalInput")
    bb_d = nc.dram_tensor("bb", [17, 4096], BF16, kind="ExternalInput")
    st_d = nc.dram_tensor("segT", [128, 512], BF16, kind="ExternalInput")
    sk_d = nc.dram_tensor("seg32", [32, 4096], BF16, kind="ExternalInput")
    out_d = nc.dram_tensor("acc", [128, 32], F32, kind="ExternalOutput")

    EXP = mybir.ActivationFunctionType.Exp
    MULT = mybir.AluOpType.mult
    MAX = mybir.AluOpType.max

    with TileContext(nc) as tc:
        with (
            tc.tile_pool(name="const", bufs=1) as constp,
            tc.tile_pool(name="kt", bufs=8) as ktp,
            tc.tile_pool(name="sc", bufs=2) as scp,
            tc.tile_pool(name="pg", bufs=3, space="PSUM") as pgp,
            tc.tile_pool(name="stk", bufs=2, space="PSUM") as stkp,
        ):
            bias_inv = constp.tile([128, 1], F32, tag="biasi")
            warm = constp.tile([128, 1], F32, tag="warm")
            ab = constp.tile([128, 512], BF16, tag="ab")
            bb = constp.tile([128, 4096], BF16, tag="bb")
            segT = constp.tile([128, 512], BF16, tag="segT")
            sg32 = constp.tile([32, 4096], BF16, tag="sg32")
            segstk = []
            for g in range(4):
                sktile = constp.tile([128, WIN], BF16, tag=f"segstk{g}")
                segstk.append(sktile)
            accT = constp.tile([128, 32], F32, tag="accT")

            # HBM loads first (~510KB total): G's deps lead on every queue.
            # bb is one quadrant's worth; it is fanned out to the other three
            # quadrants with on-device copies. seg windows are staged from
            # sg32 with 16 on-device copies spread over sync/gpsimd.
            nc.sync.dma_start(_pslice(bb, 0, 17, 0, 4096), bb_d.ap())
            nc.sync.dma_start(_pslice(ab, 0, 17, 0, 512), ab_d.ap()[0])
            nc.sync.dma_start(_pslice(ab, 32, 17, 0, 512), ab_d.ap()[1])
            nc.scalar.dma_start(_pslice(ab, 64, 17, 0, 512), ab_d.ap()[2])
            nc.scalar.dma_start(_pslice(ab, 96, 17, 0, 512), ab_d.ap()[3])
            nc.gpsimd.dma_start(sg32[:], sk_d.ap())
            # bb quadrant fan-out (waits on the bb load)
            nc.sync.dma_start(_pslice(bb, 32, 17, 0, 4096), _pslice(bb, 0, 17, 0, 4096))
            nc.scalar.dma_start(_pslice(bb, 64, 17, 0, 4096), _pslice(bb, 0, 17, 0, 4096))
            nc.gpsimd.dma_start(_pslice(bb, 96, 17, 0, 4096), _pslice(bb, 0, 17, 0, 4096))
            nc.sync.dma_start(segT[:], st_d.ap())
            nc.gpsimd.memset(accT[:], 0.0)
            nc.vector.memset(bias_inv[:], BIAS_INV)
            # one-time ACT exp-table load starts immediately
            nc.scalar.activation(warm[:], bias_inv[:], EXP, bias=bias_inv[:])
            # seg window staging: segstk[g][32q:32q+32,:] = sg32[:, 128v:128v+WIN]
            for g in range(4):
                for q in range(4):
                    v = 4 * g + q
                    eng = nc.sync if q < 2 else nc.gpsimd
                    eng.dma_start(
                        _pslice(segstk[g], 32 * q, 32, 0, WIN),
                        sg32[:, 128 * v : 128 * v + WIN],
                    )

            def emit_G(g, c):
                """4-way row-tiled G matmuls for chunk c of group g.

                Returns (kt1, kt2) after emitting converts per ASSIGN."""
                pg1 = pgp.tile([128, 1024], F32, tag="pg", name="pg1")
                pg2 = pgp.tile([128, 1024], F32, tag="pg", name="pg2")
                for q in range(4):
                    v = 4 * g + q
                    if c < 4:
                        lo = 128 * v + 512 * c
                        n = 512
                        pt = pg1 if q < 2 else pg2
                        off = 512 * (q % 2)
                    else:
                        # d16: full-partition writers must sit in distinct
                        # PSUM banks: q0/q1 -> pg1 cols 0/512, q2/q3 -> pg2
                        lo = 128 * v + 2048
                        n = 128
                        pt = pg1 if q < 2 else pg2
                        off = 512 * (q % 2)
                    nc.tensor.matmul(
                        pt[:, off : off + n],
                        _pslice(ab, 32 * q, 17, 128 * g, 128 * g + 128),
                        _pslice(bb, 32 * q, 17, lo, lo + n),
                        start=True,
                        stop=True,
                        tile_position=(32 * q, 0),
                    )
                kt1 = ktp.tile([128, 1024], BF16, tag="kt", name="kt1")
                kt2 = ktp.tile([128, 1024], BF16, tag="kt", name="kt2")
                kts = {0: (kt1, pg1), 1: (kt2, pg2)}
                for step, tile, lo, hi, eng in ASSIGN:
                    if (c < 4 and step != c) or (c == 4 and step != 4):
                        continue
                    kt, pt = kts[tile]
                    if c == 4:
                        # strided view over cols {0:128, 512:640}
                        ov = kt[:].bitcast(I16).rearrange(
                            "p (a b) -> p a b", b=512
                        )[:, :, 0:128]
                        iv = pt[:].rearrange("p (a b) -> p a b", b=512)[:, :, 0:128]
                        nc.vector.tensor_scalar(ov, iv, 0.0, None, MAX)
                        continue
                    if eng == "A":
                        nc.scalar.activation(
                            kt[:, lo:hi], pt[:, lo:hi], EXP,
                            bias=bias_inv[:], scale=S_INV,
                        )
                    elif eng == "V":
                        nc.vector.tensor_scalar(
                            kt[:, lo:hi].bitcast(I16), pt[:, lo:hi],
                            0.0, None, MAX,
                        )
                return kt1, kt2

            def emit_AS(g, c, kt1, kt2):
                """4-way column-tiled AS matmuls + DVE reduce for chunk c."""
                n = 512 if c < 4 else 128
                stk = stkp.tile([128, 512], F32, tag="stk")
                for q in range(4):
                    kt = kt1 if q < 2 else kt2
                    if c < 4:
                        rhs = kt[:, 512 * (q % 2) : 512 * (q % 2) + 512]
                        out = _pslice(stk, 32 * q, 32, 0, 512)
                    else:
                        rhs = kt[:, 512 * (q % 2) : 512 * (q % 2) + 128]
                        out = _pslice(stk, 32 * q, 32, 0, 128)
                    nc.tensor.matmul(
                        out,
                        segT[:, 32 * (4 * g + q) : 32 * (4 * g + q) + 32],
                        rhs,
                        start=True,
                        stop=True,
                        tile_position=(0, 32 * q),
                    )
                sct = scp.tile([128, 512], BF16, tag="sct")
                w0 = 512 * c if c < 4 else 2048
                if c == 0:
                    # diagonal d=0 block: half weight on cols 0:128
                    nc.vector.scalar_tensor_tensor(
                        out=sct[:, 0:128],
                        in0=stk[:, 0:128],
                        scalar=0.5,
                        in1=segstk[g][:, 0:128],
                        op0=MULT,
                        op1=MULT,
                        accum_out=accT[:, 8 * g + 5 : 8 * g + 6],
                    )
                    nc.vector.scalar_tensor_tensor(
                        out=sct[:, 128:512],
                        in0=stk[:, 128:512],
                        scalar=1.0,
                        in1=segstk[g][:, 128:512],
                        op0=MULT,
                        op1=MULT,
                        accum_out=accT[:, 8 * g : 8 * g + 1],
                    )
                else:
                    nc.vector.scalar_tensor_tensor(
                        out=sct[:, 0:n],
                        in0=stk[:, 0:n],
                        scalar=1.0 if c < 4 else 0.5,
                        in1=segstk[g][:, w0 : w0 + n],
                        op0=MULT,
                        op1=MULT,
                        accum_out=accT[:, 8 * g + c : 8 * g + c + 1],
                    )

            # Software-pipelined schedule: PE runs one G chunk ahead of AS.
            prev = None  # (g, c, kt1, kt2)
            for g in range(4):
                for c in range(5):
                    kts = emit_G(g, c)
                    if prev is not None:
                        emit_AS(prev[0], prev[1], prev[2], prev[3])
                    prev = (g, c) + kts
            emit_AS(prev[0], prev[1], prev[2], prev[3])

            nc.sync.dma_start(out_d.ap(), accT[:])
    _split_multiwait(nc)
    return nc


def _split_multiwait(nc):
    """walrus encodes at most one semaphore wait per instruction; hoist all
    but one wait onto standalone EventSemaphore instructions placed just
    before the instruction on the same engine queue."""
    ctr = 0
    for f in nc.m.functions:
        for blk in f.blocks:
            insts = blk.instructions
            out = []
            for inst in insts:
                si = inst.sync_info
                if si is not None and len(si.on_wait) > 1:
                    waits = list(si.on_wait)
                    for w in waits[:-1]:
                        es = mybir.InstEventSemaphore(
                            name=f"WSPLIT-{ctr}", ins=[], outs=[]
                        )
                        ctr += 1
                        es.engine = inst.engine
                        es.sync_info = bass_rust.SyncInfo(on_wait=[w], on_update=[])
                        out.append(es)
                    inst.sync_info = bass_rust.SyncInfo(
                        on_wait=[waits[-1]], on_update=list(si.on_update)
                    )
                out.append(inst)
            insts[:] = out


def _host_prep(images, segmentations):
    bf = ml_dtypes.bfloat16
    in_maps = []
    for cidx in range(8):
        n, h = cidx // 2, cidx % 2
        img = images[n][:, ::2, ::2]                       # nearest resize
        img = np.roll(img, -32 * h, axis=1).reshape(3, P)  # circulant rotation
        f = (img / SIGMA_RGB).astype(np.float64)
        f = f - f.mean(axis=1, keepdims=True)              # d2-invariant centering
        sq = (f * f).sum(axis=0)
        ones = np.ones((1, P), np.float64)
        b5 = np.concatenate([f, ones, (-0.5 * sq)[None]], axis=0)
        a5 = np.concatenate([f, (-0.5 * sq)[None], ones], axis=0)

        asc = A_SCALE * a5
        ah = asc.astype(bf)
        al = (asc - ah.astype(np.float64)).astype(bf)
        bh = b5.astype(bf)
        bl = (b5 - bh.astype(np.float64)).astype(bf)

        # ab[q]: rows 0-4 asc_hi, 5-9 asc_lo, 10-14 asc_hi, 15 = 127, 16 = 1
        # cols 128g..128g+127 hold row-block v=4g+q (points 128v..128v+127)
        ab = np.zeros((4, 17, 512), dtype=bf)
        for q in range(4):
            for g in range(4):
                v = 4 * g + q
                cg = slice(128 * g, 128 * g + 128)
                pv = slice(128 * v, 128 * v + 128)
                ab[q, 0:5, cg] = ah[:, pv]
                ab[q, 5:10, cg] = al[:, pv]
                ab[q, 10:15, cg] = ah[:, pv]
            ab[q, 15, :] = bf(127.0)
            ab[q, 16, :] = bf(1.0)
        # bb[q]: rows 0-4 b_hi, 5-9 b_hi, 10-14 b_lo, 15 = 128, 16 = -7
        bb = np.zeros((17, 4096), dtype=bf)
        bb[0:5] = bh
        bb[5:10] = bh
        bb[10:15] = bl
        bb[15, :] = bf(128.0)
        bb[16, :] = bf(C_ADJ)

        # seg: roll + 2x2 sum pool (/16 folded into final host scale)
        segr = np.roll(segmentations[n], -64 * h, axis=1).astype(np.float64)
        sp = segr.reshape(21, 64, 2, 64, 2).sum(axis=(2, 4)).reshape(21, P)
        spb = sp.astype(bf)

        segT = np.zeros((128, 512), dtype=bf)
        sT = segT.reshape(128, 16, 32)
        for v in range(16):
            sT[:, v, 0:21] = spb[:, 128 * v : 128 * v + 128].T
        sg32 = np.zeros((32, 4096), dtype=bf)
        sg32[0:21] = spb

        in_maps.append(
            {
                "ab": ab,
                "bb": bb,
                "segT": np.ascontiguousarray(segT),
                "seg32": sg32,
            }
        )
    return in_maps


def run(images, segmentations, trace=False):
    if "nc" not in _CACHED:
        _CACHED["nc"] = _build_nc()
    nc = _CACHED["nc"]
    in_maps = _host_prep(np.asarray(images), np.asarray(segmentations))
    res = run_bass_kernel_spmd(nc, in_maps, list(range(8)), trace=trace)
    total = np.float64(0.0)
    for r in res.results:
        acc = r["acc"].astype(np.float64)
        for g in range(4):
            total += acc[:, 8 * g : 8 * g + 6].sum()
    # x2 symmetric halves, /16 unscaled 2x2 pool (quadratic), -W, /N batch mean
    loss = -WEIGHT * 2.0 * total / 16.0 / N_IMG
    return np.array([loss], dtype=np.float32), res


def kernel(images, segmentations):
    out, _ = run(images, segmentations, trace=False)
    return out


# revision 12
# speedup vs baseline: 1.1225x; 1.1216x over previous
"""ColorDenseCRFLoss on 8 Trainium2 NeuronCores — 3-engine exp redesign.

Math: loss = -W/N * sum_n sum_ij K_ij S_ij, where for each image n
  K_ij = exp(-0.5*||f_i - f_j||^2)   (f = nearest-downsampled RGB / 15, P=4096 pts)
  S_ij = sum_k seg_k,i seg_k,j       (seg = bilinear-downsampled softmax, K=21)
Bilinear downsample at exactly 2x == 2x2 average pooling; nearest == stride-2.

Sharding: 2 cores per image (batch N=4 -> 8 cores). Symmetry via circulant
blocks: core h of image n owns row-blocks v=0..15 (of its rotated frame,
rotation 2048*h points) with column window d=0..16 (2176 cols); d=0 and d=16
columns are weighted 1/2 (folded into host-side segstk scaling) and the
grand total is doubled.

The PE emits pG = A*g + B' (A=128/ln2, B'=127*128-7; scale/bias folded in as
two extra contraction rows), so THREE engines convert PSUM tiles to K in
parallel:
 - ACT: exact exp via activation(Exp, scale=ln2/128, bias=-B'*ln2/128)
 - DVE: Schraudolph bf16: int16(max(pG,0)) bitcast as bf16 (RNE + saturation)
 - GpSimd: same via tensor_tensor(max vs zero tile) (tensor_scalar is slow on Q7)
The Schraudolph bias constant (-7) is calibrated so the kernel-weighted sum
error is ~3e-4 even if ALL elements used it; only ~half do.

G and AS matmuls are 4-way PE-tiled (row-tiles for G at partition offsets
0/32/64/96; column-tiles for AS at psum partition offsets) — all four streams
run concurrently on the PE. All seg staging (2x2 pool, transpose, window
stacking, edge halving) is done host-side; the device only does matmuls,
converts, and the DVE multiply-accumulate reduce.
"""

import sys

for _p in ("/opt/trn_rl_repo",):
    if _p not in sys.path:
        sys.path.insert(0, _p)

import numpy as np
import ml_dtypes

import bass_rust
import concourse.bass as bass
import concourse.mybir as mybir
from concourse.tile import TileContext
from concourse.bass_utils import run_bass_kernel_spmd

F32 = mybir.dt.float32
BF16 = mybir.dt.bfloat16
I16 = mybir.dt.int16

WEIGHT = 1e-7
SIGMA_RGB = 15.0
N_IMG = 4
P = 4096          # 64*64 points per image
WIN = 17 * 128    # d = 0..16 column window (2176)

LN2 = float(np.log(2.0))
A_SCALE = 128.0 / LN2            # Schraudolph exponent scale
C_ADJ = -7.0                     # chord-bias correction (calibrated)
B_BIAS = 127.0 * 128.0 + C_ADJ   # 16249
S_INV = LN2 / 128.0              # ACT inverse scale
BIAS_INV = -B_BIAS * LN2 / 128.0  # ACT inverse bias

# Per-group convert assignment: (step, tile, lo, hi, engine)
#   step: 0..3 = 512-col chunks, 4 = d16 (512-wide packed tile)
#   tile: 0 = kt1 (row-blocks q0/q1), 1 = kt2 (q2/q3); d16 has a single tile
#   engine: 'A' = ACT exact exp, 'V' = DVE schraudolph, 'G' = gpsimd schraudolph
ASSIGN = [
    (0, 0, 0, 1024, "A"),
    (0, 1, 0, 1024, "A"),
    (1, 0, 0, 1024, "A"),
    (1, 1, 0, 1024, "A"),
    (2, 0, 0, 1024, "A"),
    (2, 1, 0, 1024, "A"),
    (3, 0, 0, 1024, "V"),
    (3, 1, 0, 1024, "V"),
    (4, 0, 0, 0, "V"),
    (4, 1, 0, 0, "V"),
]

_CACHED = {}


def _pslice(t, lo, n, c0, c1):
    # [lo:lo+n, c0:c1] partition+col slice; base 96 must be expressed as a
    # double-slice (AP base_partition rejects 96).
    if lo >= 96:
        return t[64:128, c0:c1][lo - 64 : lo - 64 + n, :]
    return t[lo : lo + n, c0:c1]


def _build_nc():
    nc = bass.Bass(trn_type="TRN2", target_bir_lowering=False, debug=False)
    ab_d = nc.dram_tensor("ab", [128, 512], BF16, kind="ExternalInput")
    bb_d = nc.dram_tensor("bb", [4, 17, 4096], BF16, kind="ExternalInput")
    st_d = nc.dram_tensor("segT", [128, 512], BF16, kind="ExternalInput")
    sk_d = nc.dram_tensor("segstk", [4, 128, WIN], BF16, kind="ExternalInput")
    out_d = nc.dram_tensor("acc", [128, 32], F32, kind="ExternalOutput")

    EXP = mybir.ActivationFunctionType.Exp
    MULT = mybir.AluOpType.mult
    MAX = mybir.AluOpType.max

    with TileContext(nc) as tc:
        with (
            tc.tile_pool(name="const", bufs=1) as constp,
            tc.tile_pool(name="kt", bufs=8) as ktp,
            tc.tile_pool(name="sc", bufs=2) as scp,
            tc.tile_pool(name="pg", bufs=3, space="PSUM") as pgp,
            tc.tile_pool(name="stk", bufs=2, space="PSUM") as stkp,
        ):
            bias_inv = constp.tile([128, 1], F32, tag="biasi")
            warm = constp.tile([128, 1], F32, tag="warm")
            ab = constp.tile([128, 512], BF16, tag="ab")
            bb = constp.tile([128, 4096], BF16, tag="bb")
            segT = constp.tile([128, 512], BF16, tag="segT")
            segstk = []
            for g in range(4):
                sktile = constp.tile([128, WIN], BF16, tag=f"segstk{g}")
                segstk.append(sktile)
            accT = constp.tile([128, 32], F32, tag="accT")

            # HBM loads: one DMA instruction streams on ONE ring
            # (~22 GB/s + ~3.5us fixed latency), so every large tensor is
            # split into column-part DMAs that overlap across rings.
            # Priority-ordered round-robin across the three queues; the
            # scalar (ACT) queue gets only the first few so converts start
            # early.
            dmas = []
            for q in range(4):   # bb low cols: G groups 0-1 + part of 2
                dmas.append((_pslice(bb, 32 * q, 17, 0, 2048), bb_d.ap()[q][:, 0:2048]))
            dmas.append((ab[:, 0:128], ab_d.ap()[:, 0:128]))
            dmas.append((segT[:, 0:256], st_d.ap()[:, 0:256]))
            dmas.append((_pslice(segstk[0], 0, 128, 0, 544), sk_d.ap()[0][:, 0:544]))
            dmas.append((_pslice(segstk[0], 0, 128, 544, 1088), sk_d.ap()[0][:, 544:1088]))
            dmas.append((ab[:, 128:512], ab_d.ap()[:, 128:512]))
            dmas.append((segT[:, 256:512], st_d.ap()[:, 256:512]))
            for q in range(4):   # bb high cols
                dmas.append((_pslice(bb, 32 * q, 17, 2048, 4096), bb_d.ap()[q][:, 2048:4096]))
            dmas.append((_pslice(segstk[0], 0, 128, 1088, 1632), sk_d.ap()[0][:, 1088:1632]))
            dmas.append((_pslice(segstk[0], 0, 128, 1632, WIN), sk_d.ap()[0][:, 1632:WIN]))
            for g in (1, 2, 3):
                dmas.append((_pslice(segstk[g], 0, 128, 0, 1088), sk_d.ap()[g][:, 0:1088]))
                dmas.append((_pslice(segstk[g], 0, 128, 1088, WIN), sk_d.ap()[g][:, 1088:WIN]))
            queues = [nc.sync, nc.scalar, nc.gpsimd]
            scalar_budget = 5
            qi = 0
            for dst, srcap in dmas:
                eng = queues[qi % 3]
                if eng is nc.scalar:
                    if scalar_budget == 0:
                        qi += 1
                        eng = queues[qi % 3]
                    else:
                        scalar_budget -= 1
                eng.dma_start(dst, srcap)
                qi += 1
            nc.gpsimd.memset(accT[:], 0.0)
            nc.vector.memset(bias_inv[:], BIAS_INV)
            # one-time ACT exp-table load starts immediately
            nc.scalar.activation(warm[:], bias_inv[:], EXP, bias=bias_inv[:])

            def emit_G(g, c):
                """4-way row-tiled G matmuls for chunk c of group g.

                Returns (kt1, kt2) after emitting converts per ASSIGN."""
                pg1 = pgp.tile([128, 1024], F32, tag="pg", name="pg1")
                pg2 = pgp.tile([128, 1024], F32, tag="pg", name="pg2")
                for q in range(4):
                    v = 4 * g + q
                    if c < 4:
                        lo = 128 * v + 512 * c
                        n = 512
                        pt = pg1 if q < 2 else pg2
                        off = 512 * (q % 2)
                    else:
                        # d16: full-partition writers must sit in distinct
                        # PSUM banks: q0/q1 -> pg1 cols 0/512, q2/q3 -> pg2
                        lo = 128 * v + 2048
                        n = 128
                        pt = pg1 if q < 2 else pg2
                        off = 512 * (q % 2)
                    nc.tensor.matmul(
                        pt[:, off : off + n],
                        _pslice(ab, 32 * q, 17, 128 * g, 128 * g + 128),
                        _pslice(bb, 32 * q, 17, lo, lo + n),
                        start=True,
                        stop=True,
                        tile_position=(32 * q, 0),
                    )
                kt1 = ktp.tile([128, 1024], BF16, tag="kt", name="kt1")
                kt2 = ktp.tile([128, 1024], BF16, tag="kt", name="kt2")
                kts = {0: (kt1, pg1), 1: (kt2, pg2)}
                for step, tile, lo, hi, eng in ASSIGN:
                    if (c < 4 and step != c) or (c == 4 and step != 4):
                        continue
                    kt, pt = kts[tile]
                    if c == 4:
                        # strided view over cols {0:128, 512:640}
                        ov = kt[:].bitcast(I16).rearrange(
                            "p (a b) -> p a b", b=512
                        )[:, :, 0:128]
                        iv = pt[:].rearrange("p (a b) -> p a b", b=512)[:, :, 0:128]
                        nc.vector.tensor_scalar(ov, iv, 0.0, None, MAX)
                        continue
                    if eng == "A":
                        nc.scalar.activation(
                            kt[:, lo:hi], pt[:, lo:hi], EXP,
                            bias=bias_inv[:], scale=S_INV,
                        )
                    elif eng == "V":
                        nc.vector.tensor_scalar(
                            kt[:, lo:hi].bitcast(I16), pt[:, lo:hi],
                            0.0, None, MAX,
                        )
                return kt1, kt2

            def emit_AS(g, c, kt1, kt2):
                """4-way column-tiled AS matmuls + DVE reduce for chunk c."""
                n = 512 if c < 4 else 128
                stk = stkp.tile([128, 512], F32, tag="stk")
                for q in range(4):
                    kt = kt1 if q < 2 else kt2
                    if c < 4:
                        rhs = kt[:, 512 * (q % 2) : 512 * (q % 2) + 512]
                        out = _pslice(stk, 32 * q, 32, 0, 512)
                    else:
                        rhs = kt[:, 512 * (q % 2) : 512 * (q % 2) + 128]
                        out = _pslice(stk, 32 * q, 32, 0, 128)
                    nc.tensor.matmul(
                        out,
                        segT[:, 32 * (4 * g + q) : 32 * (4 * g + q) + 32],
                        rhs,
                        start=True,
                        stop=True,
                        tile_position=(0, 32 * q),
                    )
                sct = scp.tile([128, 512], BF16, tag="sct")
                w0 = 512 * c if c < 4 else 2048
                if c == 0:
                    # diagonal d=0 block: half weight on cols 0:128
                    nc.vector.scalar_tensor_tensor(
                        out=sct[:, 0:128],
                        in0=stk[:, 0:128],
                        scalar=0.5,
                        in1=segstk[g][:, 0:128],
                        op0=MULT,
                        op1=MULT,
                        accum_out=accT[:, 8 * g + 5 : 8 * g + 6],
                    )
                    nc.vector.scalar_tensor_tensor(
                        out=sct[:, 128:512],
                        in0=stk[:, 128:512],
                        scalar=1.0,
                        in1=segstk[g][:, 128:512],
                        op0=MULT,
                        op1=MULT,
                        accum_out=accT[:, 8 * g : 8 * g + 1],
                    )
                else:
                    nc.vector.scalar_tensor_tensor(
                        out=sct[:, 0:n],
                        in0=stk[:, 0:n],
                        scalar=1.0 if c < 4 else 0.5,
                        in1=segstk[g][:, w0 : w0 + n],
                        op0=MULT,
                        op1=MULT,
                        accum_out=accT[:, 8 * g + c : 8 * g + c + 1],
                    )

            # Software-pipelined schedule: PE runs one G chunk ahead of AS.
            prev = None  # (g, c, kt1, kt2)
            for g in range(4):
                for c in range(5):
                    kts = emit_G(g, c)
                    if prev is not None:
                        emit_AS(prev[0], prev[1], prev[2], prev[3])
                    prev = (g, c) + kts
            emit_AS(prev[0], prev[1], prev[2], prev[3])

            nc.sync.dma_start(out_d.ap(), accT[:])
    _split_multiwait(nc)
    return nc


def _split_multiwait(nc):
    """walrus encodes at most one semaphore wait per instruction; hoist all
    but one wait onto standalone EventSemaphore instructions placed just
    before the instruction on the same engine queue."""
    ctr = 0
    for f in nc.m.functions:
        for blk in f.blocks:
            insts = blk.instructions
            out = []
            for inst in insts:
                si = inst.sync_info
                if si is not None and len(si.on_wait) > 1:
                    waits = list(si.on_wait)
                    for w in waits[:-1]:
                        es = mybir.InstEventSemaphore(
                            name=f"WSPLIT-{ctr}", ins=[], outs=[]
                        )
                        ctr += 1
                        es.engine = inst.engine
                        es.sync_info = bass_rust.SyncInfo(on_wait=[w], on_update=[])
                        out.append(es)
                    inst.sync_info = bass_rust.SyncInfo(
                        on_wait=[waits[-1]], on_update=list(si.on_update)
                    )
                out.append(inst)
            insts[:] = out


def _host_prep(images, segmentations):
    bf = ml_dtypes.bfloat16
    in_maps = []
    for cidx in range(8):
        n, h = cidx // 2, cidx % 2
        img = images[n][:, ::2, ::2]                       # nearest resize
        img = np.roll(img, -32 * h, axis=1).reshape(3, P)  # circulant rotation
        f = (img / SIGMA_RGB).astype(np.float64)
        f = f - f.mean(axis=1, keepdims=True)              # d2-invariant centering
        sq = (f * f).sum(axis=0)
        ones = np.ones((1, P), np.float64)
        b5 = np.concatenate([f, ones, (-0.5 * sq)[None]], axis=0)
        a5 = np.concatenate([f, (-0.5 * sq)[None], ones], axis=0)

        asc = A_SCALE * a5
        ah = asc.astype(bf)
        al = (asc - ah.astype(np.float64)).astype(bf)
        bh = b5.astype(bf)
        bl = (b5 - bh.astype(np.float64)).astype(bf)

        # ab[q]: rows 0-4 asc_hi, 5-9 asc_lo, 10-14 asc_hi, 15 = 127, 16 = 1
        # cols 128g..128g+127 hold row-block v=4g+q (points 128v..128v+127)
        ab = np.zeros((128, 512), dtype=bf)
        for q in range(4):
            for g in range(4):
                v = 4 * g + q
                cg = slice(128 * g, 128 * g + 128)
                pv = slice(128 * v, 128 * v + 128)
                ab[32 * q + 0 : 32 * q + 5, cg] = ah[:, pv]
                ab[32 * q + 5 : 32 * q + 10, cg] = al[:, pv]
                ab[32 * q + 10 : 32 * q + 15, cg] = ah[:, pv]
            ab[32 * q + 15, :] = bf(127.0)
            ab[32 * q + 16, :] = bf(1.0)
        # bb[q]: rows 0-4 b_hi, 5-9 b_hi, 10-14 b_lo, 15 = 128, 16 = -7
        b17 = np.zeros((17, 4096), dtype=bf)
        b17[0:5] = bh
        b17[5:10] = bh
        b17[10:15] = bl
        b17[15, :] = bf(128.0)
        b17[16, :] = bf(C_ADJ)
        bb = np.ascontiguousarray(np.broadcast_to(b17, (4, 17, 4096)))

        # seg: roll + 2x2 sum pool (/16 folded into final host scale)
        segr = np.roll(segmentations[n], -64 * h, axis=1).astype(np.float64)
        sp = segr.reshape(21, 64, 2, 64, 2).sum(axis=(2, 4)).reshape(21, P)
        spb = sp.astype(bf)

        segT = np.zeros((128, 512), dtype=bf)
        sT = segT.reshape(128, 16, 32)
        for v in range(16):
            sT[:, v, 0:21] = spb[:, 128 * v : 128 * v + 128].T
        segstk = np.zeros((4, 128, WIN), dtype=bf)
        for g in range(4):
            for q in range(4):
                v = 4 * g + q
                segstk[g, 32 * q : 32 * q + 21, :] = spb[:, 128 * v : 128 * v + WIN]

        in_maps.append(
            {
                "ab": ab,
                "bb": bb,
                "segT": np.ascontiguousarray(segT),
                "segstk": segstk,
            }
        )
    return in_maps


def run(images, segmentations, trace=False):
    if "nc" not in _CACHED:
        _CACHED["nc"] = _build_nc()
    nc = _CACHED["nc"]
    in_maps = _host_prep(np.asarray(images), np.asarray(segmentations))
    res = run_bass_kernel_spmd(nc, in_maps, list(range(8)), trace=trace)
    total = np.float64(0.0)
    for r in res.results:
        acc = r["acc"].astype(np.float64)
        for g in range(4):
            total += acc[:, 8 * g : 8 * g + 6].sum()
    # x2 symmetric halves, /16 unscaled 2x2 pool (quadratic), -W, /N batch mean
    loss = -WEIGHT * 2.0 * total / 16.0 / N_IMG
    return np.array([loss], dtype=np.float32), res


def kernel(images, segmentations):
    out, _ = run(images, segmentations, trace=False)
    return out
